# revision 40
# baseline (speedup 1.0000x reference)
"""Fused attention layer (QKV projections + softmax(QK^T/sqrt(d))V) for
Trainium2, data-parallel over the batch across 8 NeuronCores.

Projection-free formulation (per core, one batch element, S=4096, D=512):
  scores^T = key (Wk^T Wq) query^T + v[k] 1^T + 1 u[q]^T (+ const); the
  per-query additive terms cancel in softmax, so only the per-key bias
  v = key (Wk^T bq) survives and rides through the ACT exp's per-partition
  bias.  With G = Wk^T Wq folded on the HOST and applied to the key side
  (KG^T = G^T key^T), the query projection disappears entirely.  On the
  value side, out = attn value Wv^T + bv (attn rows sum to one), so value
  is consumed in its natural layout; U^T = value^T exp^T accumulates on
  PSUM in two e-chunk passes, and Wv^T is applied per 128-query tile at
  the end.  The bias enters as rowsum[q]*bv via a K=1 matmul so the final
  1/rowsum ACT scaling leaves exactly +bv.

This version removes nearly all non-GEMM work from the PE:
  - query/key/value are staged as fp16 in DRAM (host cast, identical
    rounding to the previous on-device cast); query^T is delivered by the
    DMA XBAR transpose (sync queue only, <=2048-row calls, token-fenced
    from all normal DMAs -- concurrent/cross-queue use corrupts), key^T
    by PE transposes off an early natural-layout key load, pipelined with
    the KG matmuls.
  - G = Wk^T Wq is folded on the host (weights-only prep).
  - the per-key bias v = key @ (scale Wk^T bq) is a GPSIMD multiply plus
    DVE free-dim reduce over the natural-layout key copy.
  - the output bias bv is a DVE broadcast-add (attn rows sum to one), so
    no K=1 bias matmuls except on the final block where the rowsum*bv
    psum trick keeps the tail chain PE-resident.
  - the softmax reciprocal runs after the PE transpose in [128,1] layout
    (128-lane parallel) instead of on a single partition.
All matmul operands fp16 (1 cyc/row, ~6e-4 rel err), accumulation fp32.
"""

import math

import numpy as np

S, D, P = 4096, 512, 128
NCORES = 8
KB = 512  # q block width


def build_attention(s=S, d=D, num_devices=NCORES):
    from contextlib import ExitStack

    import concourse.mybir as mybir
    import concourse.tile as tile
    from concourse import bacc
    from concourse.masks import make_identity

    f32 = mybir.dt.float32
    f16 = mybir.dt.float16
    Act = mybir.ActivationFunctionType
    Alu = mybir.AluOpType

    dc = d // P        # d/e chunks (4)
    nkc = s // P       # key chunks (32)
    nqb = s // KB      # q blocks (8)
    tpb = KB // P      # 128-sub-blocks per block (4)
    nh = 2 if nqb >= 2 else 1  # sequence halves (XBAR <=2048-row calls)
    sh = s // nh
    bph = nqb // nh    # q blocks per half
    kph = nkc // nh    # key chunks per half
    softmax_scale = 1.0 / math.sqrt(d)

    nc = bacc.Bacc(
        "TRN2", target_bir_lowering=False, debug=False, num_devices=num_devices
    )

    q16_d = nc.dram_tensor("q16", [s, d], f16, kind="ExternalInput").ap()
    k16_d = nc.dram_tensor("k16", [s, d], f16, kind="ExternalInput").ap()
    v16_d = nc.dram_tensor("v16", [s, d], f16, kind="ExternalInput").ap()
    g16_d = nc.dram_tensor("g16", [d, d], f16, kind="ExternalInput").ap()
    wvt_d = nc.dram_tensor("wvt", [d, d], f16, kind="ExternalInput").ap()
    w1b_d = nc.dram_tensor("w1b", [P, d], f16, kind="ExternalInput").ap()
    bv_d = nc.dram_tensor("bv128", [P, d], f16, kind="ExternalInput").ap()
    out_d = nc.dram_tensor("out", [s, d], f32, kind="ExternalOutput").ap()

    with tile.TileContext(nc) as tc, ExitStack() as stack:
        consts = stack.enter_context(tc.tile_pool(name="consts", bufs=1))

        one16 = consts.tile([1, 1], f16, name="one16")
        nc.vector.memset(one16, 1.0)
        ident16 = consts.tile([P, P], f16, name="ident16")
        make_identity(nc, ident16)
        ones_col = consts.tile([P, 1], f16, name="ones_col")
        nc.vector.memset(ones_col, 1.0)

        gsb = consts.tile([P, dc, d], f16, name="g_sb")       # G = Wk^T Wq
        wvt = consts.tile([P, dc, d], f16, name="wvt_sb")
        w1b = consts.tile([P, d], f16, name="w1b_sb")         # bcast scale*Wk^T bq
        bv128 = consts.tile([P, d], f16, name="bv128_sb")     # bcast bv

        # persistent activations (query^T / value split into sequence-half
        # tiles for finer DMA->compute dependency granularity)
        qryt_h = [
            consts.tile([P, dc, sh], f16, name=f"qryt_sb{i}") for i in range(nh)
        ]
        kgt = consts.tile([P, dc, s], f16, name="kgt_sb")     # (key G)^T [d', n]
        vnat_h = [
            consts.tile([P, kph, d], f16, name=f"vnat_sb{i}") for i in range(nh)
        ]
        vb = consts.tile([P, nkc], f32, name="vb_sb")         # scale * key@w1

        ps_st = stack.enter_context(tc.tile_pool(name="ps_st", bufs=2, space="PSUM"))

        # ---------------- Phase 1: loads + KG^T (keyt/knat transient) ----------
        with (
            tc.tile_pool(name="keyt_pool", bufs=1) as keyt_pool,
            tc.tile_pool(name="knat_pool", bufs=1) as knat_pool,
            tc.tile_pool(name="vbs_pool", bufs=1) as vbs_pool,
            tc.tile_pool(name="pt_ps", bufs=4, space="PSUM") as pt_ps,
            tc.tile_pool(name="kg_ps2", bufs=2, space="PSUM") as kg_ps2,
        ):
            knat = knat_pool.tile([P, nkc, d], f16, name="knat_sb")
            scratch2 = [
                vbs_pool.tile([P, d], f16, name=f"vb_scratch{i}") for i in range(8)
            ]
            keyt_h = [
                keyt_pool.tile([P, dc, sh], f16, name=f"keyt_sb{i}")
                for i in range(nh)
            ]

            # XBAR DMA-transpose discipline (hardware bug workarounds, verified
            # by probing): XBAR transposes corrupt when they execute
            # concurrently with normal DMAs (either queue) or with transposes
            # on the other queue.  So: key^T is built by PE transposes from an
            # early natural-layout key load, and only query^T (8 calls, <=2048
            # source rows each) uses the XBAR -- all on the sync queue, with
            # tiny DVE "token" copies creating data dependencies that keep
            # every normal DMA strictly outside the transpose windows.

            # group 1: natural-layout key (first pieces first, so the PE keyt
            # transposes start as early as possible) + the weights needed
            # before the epilogue; wvt is deferred to the value group
            pieces = [(0, 1), (1, 2), (2, 3), (3, 4), (4, 6), (6, 8)]  # eighths of s
            e8 = nkc // 8 if nkc >= 8 else 1
            pieces = [(a * e8, b * e8) for a, b in pieces if b * e8 <= nkc]
            if pieces[-1][1] < nkc:
                pieces.append((pieces[-1][1], nkc))
            nc.sync.dma_start(
                out=knat[:, pieces[0][0] : pieces[0][1], :],
                in_=k16_d[pieces[0][0] * P : pieces[0][1] * P, :].rearrange(
                    "(s p) d -> p s d", p=P
                ),
            )
            nc.scalar.dma_start(
                out=knat[:, pieces[1][0] : pieces[1][1], :],
                in_=k16_d[pieces[1][0] * P : pieces[1][1] * P, :].rearrange(
                    "(s p) d -> p s d", p=P
                ),
            )
            nc.sync.dma_start(out=gsb, in_=g16_d.rearrange("(c p) e -> p c e", p=P))
            for j, (a, b) in enumerate(pieces[2:]):
                eng = nc.sync if j % 2 == 0 else nc.scalar
                eng.dma_start(
                    out=knat[:, a:b, :],
                    in_=k16_d[a * P : b * P, :].rearrange("(s p) d -> p s d", p=P),
                )
            # w1b/bv128 are not needed until the vb reduces / epilogue: keep
            # them behind the key pieces so the transpose pipeline never
            # starves on aggregate DMA bandwidth
            nc.scalar.dma_start(out=w1b, in_=w1b_d)
            nc.scalar.dma_start(out=bv128, in_=bv_d)
            # token A: the qryt h0 XBAR transposes wait for all group-1 DMAs.
            # Tokens are 8-byte SBUF->SBUF micro-DMAs on the sync queue (the
            # queue the XBARs live on): a token on a compute engine becomes a
            # head-of-line block for everything paced behind it.
            qtok = qryt_h[0][0:1, :, 0:2]
            nc.sync.dma_start(out=qtok, in_=gsb[0:1, :, 0:2])
            nc.sync.dma_start(out=qtok, in_=w1b[0:1, 0:8])
            nc.sync.dma_start(out=qtok, in_=bv128[0:1, 0:8])
            for a, b in pieces:
                nc.sync.dma_start(out=qtok, in_=knat[0:1, a : a + 1, 0:8])
            # group 2: query^T first half on the XBAR (sync queue only)
            for c in range(dc):
                nc.sync.dma_start(
                    out=qryt_h[0][:, c, :],
                    in_=q16_d[0:sh, c * P : (c + 1) * P],
                    transpose=True,
                )
            # token B: natural loads wait for the qryt h0 transposes
            for i in range(nh):
                nc.sync.dma_start(
                    out=vnat_h[i][0:1, 0:4, 0:2], in_=qryt_h[0][0:1, :, 0:2]
                )
            nc.sync.dma_start(out=wvt[0:1, :, 0:2], in_=qryt_h[0][0:1, :, 0:2])
            # group 3: natural-layout value + deferred Wv^T weight.  All on
            # the SYNC queue: a gated DMA-issue on the scalar queue would
            # block the ACT engine's copy/drain/exp stream (one FIFO per
            # engine), which stalled the scores pipeline by ~8us.
            for i in range(nh):
                nc.sync.dma_start(
                    out=vnat_h[i],
                    in_=v16_d[i * sh : (i + 1) * sh, :].rearrange(
                        "(s p) d -> p s d", p=P
                    ),
                )
            nc.sync.dma_start(out=wvt, in_=wvt_d.rearrange("(c p) e -> p c e", p=P))
            if nh > 1:
                # token C: qryt h1 transposes wait for the natural loads
                for i in range(nh):
                    nc.sync.dma_start(
                        out=qryt_h[1][0:1, :, 0:2], in_=vnat_h[i][0:1, 0:4, 0:2]
                    )
                nc.sync.dma_start(
                    out=qryt_h[1][0:1, :, 0:2], in_=wvt[0:1, :, 0:2]
                )
                # group 4: query^T second half on the XBAR
                for c in range(dc):
                    nc.sync.dma_start(
                        out=qryt_h[1][:, c, :],
                        in_=q16_d[sh:s, c * P : (c + 1) * P],
                        transpose=True,
                    )

            # key^T per block via PE transposes from knat (psum -> ACT copy),
            # software-pipelined one block ahead of the KG matmuls; the DVE
            # v-vector reduces v[k] = key @ (scale Wk^T bq) trail each block
            cpb = nkc // nqb  # key chunks per q-sized block (4)

            def emit_keyt_block(nb):
                # two 128-key sub-blocks per psum tile: halves the copy
                # instruction count and the per-block cross-engine latency
                # in the transpose -> copy -> KG pipeline
                kh, nboff = divmod(nb, bph)
                for pi in range(cpb // 2):
                    pt = pt_ps.tile([P, dc, 2 * P], f16, tag="pt")
                    for sp in range(2):
                        si = 2 * pi + sp
                        for c in range(dc):
                            nc.tensor.transpose(
                                pt[:, c, sp * P : (sp + 1) * P],
                                knat[:, nb * cpb + si, c * P : (c + 1) * P],
                                ident16,
                            )
                    a = (nboff * cpb + 2 * pi) * P
                    dst = keyt_h[kh][:, :, a : a + 2 * P]
                    if pi == 1:
                        nc.vector.tensor_copy(out=dst, in_=pt)
                    else:
                        nc.scalar.copy(out=dst, in_=pt)

            def emit_kg_block(nb):
                # the last two blocks use a dedicated psum pool so the scores
                # pipeline's psum_st slots never wait on late KG drains
                pool, tag = (
                    (kg_ps2, "kg2") if nb >= nqb - 2 else (ps_st, "psum_st")
                )
                kh, nboff = divmod(nb, bph)
                for ec in range(dc):
                    pp = pool.tile([P, KB], f32, tag=tag)
                    for c in range(dc):
                        nc.tensor.matmul(
                            pp,
                            gsb[:, c, ec * P : (ec + 1) * P],
                            keyt_h[kh][:, c, nboff * KB : (nboff + 1) * KB],
                            start=(c == 0),
                            stop=(c == dc - 1),
                        )
                    if ec % 2:
                        nc.scalar.copy(
                            out=kgt[:, ec, nb * KB : (nb + 1) * KB], in_=pp
                        )
                    else:
                        nc.vector.tensor_copy(
                            out=kgt[:, ec, nb * KB : (nb + 1) * KB], in_=pp
                        )
            def emit_vb(i):
                scratch = scratch2[i % 8]
                nc.gpsimd.tensor_mul(scratch, knat[:, i, :], w1b)
                nc.vector.tensor_reduce(
                    out=vb[:, i : i + 1],
                    in_=scratch,
                    axis=mybir.AxisListType.X,
                    op=Alu.add,
                )

            # v[k] = key @ (scale Wk^T bq): multiply on the (otherwise idle)
            # GPSIMD, free-dim reduce on the DVE, one chunk per KG block with
            # the remainder after the loop
            emit_keyt_block(0)
            for nb in range(1, nqb):
                emit_keyt_block(nb)
                emit_kg_block(nb - 1)
                emit_vb(nb - 1)
            emit_kg_block(nqb - 1)
            for i in range(nqb - 1, nkc):
                emit_vb(i)

        # ---------------- Phase 2: attention (scores transposed) ----------------
        with (
            tc.tile_pool(name="ps_small", bufs=2, space="PSUM") as ps_small,
            tc.tile_pool(name="expt_pool", bufs=nkc) as expt_pool,
            tc.tile_pool(name="rsum_pool", bufs=2) as rsum_pool,
            tc.tile_pool(name="unsb_pool", bufs=2) as unsb_pool,
            tc.tile_pool(name="osb_pool", bufs=2) as osb_pool,
            tc.tile_pool(name="stat_pool", bufs=8) as stat_pool,
            tc.tile_pool(name="ps_ut", bufs=2, space="PSUM") as ps_ut,
        ):

            def emit_output(qb, un_sb, rs16, rcs):
                for qs in range(tpb):
                    last = qb == nqb - 1
                    po = ps_small.tile([P, d], f32, tag="ps_small")
                    for c in range(dc):
                        nc.tensor.matmul(
                            po,
                            un_sb[:, c, qs * P : (qs + 1) * P],
                            wvt[:, c, :],
                            start=(c == 0),
                            stop=(c == dc - 1) and not last,
                        )
                    if last:
                        # rowsum[q]*bv rides the psum so the final 1/rowsum
                        # scaling leaves exactly +bv with no DVE op in the
                        # critical tail chain
                        nc.tensor.matmul(
                            po,
                            rs16[0:1, qs * P : (qs + 1) * P],
                            bv128[0:1, :],
                            start=False,
                            stop=True,
                        )
                    out_sb = osb_pool.tile([P, d], f32, tag="out_sb")
                    if qb == 0 and qs == 0:
                        # token D: the first output DMA (a normal DMA) must not
                        # overlap the trailing qryt transposes
                        nc.gpsimd.tensor_copy(
                            out=out_sb[0:1, 0:2],
                            in_=qryt_h[nh - 1][0:1, 0:1, sh - 2 : sh],
                        )
                    # out = po/rowsum + bv (attn rows sum to one, so the bias
                    # needs no rowsum compensation): scale on ACT, bias on DVE
                    if last and qs % 2:
                        nc.vector.tensor_scalar_mul(out_sb, po, rcs[qs][:, 0:1])
                    else:
                        nc.scalar.activation(
                            out=out_sb,
                            in_=po,
                            func=Act.Identity,
                            scale=rcs[qs][:, 0:1],
                        )
                    if not last:
                        nc.vector.tensor_add(out_sb, out_sb, bv128)
                    oeng = nc.scalar if last and qs % 2 else nc.sync
                    oeng.dma_start(
                        out=out_d[qb * KB + qs * P : qb * KB + (qs + 1) * P, :],
                        in_=out_sb,
                    )

            pending = None
            for qb in range(nqb):
                rsum = rsum_pool.tile([P, KB], f32, tag="rsum")
                ut_a = ps_ut.tile([P, 2, KB], f32, tag="ut")
                un_sb = unsb_pool.tile([P, dc, KB], f16, tag="un_sb")
                expts = []
                for kc in range(nkc):
                    psum_st = ps_st.tile([P, KB], f32, tag="psum_st")
                    qh, qoff = divmod(qb, bph)
                    for ec in range(dc):
                        nc.tensor.matmul(
                            psum_st,
                            kgt[:, ec, kc * P : (kc + 1) * P],
                            qryt_h[qh][:, ec, qoff * KB : (qoff + 1) * KB],
                            start=(ec == 0),
                            stop=(ec == dc - 1),
                        )
                    expt = expt_pool.tile([P, KB], f16, tag="expt")
                    expts.append(expt)
                    nc.scalar.activation(
                        out=expt,
                        in_=psum_st,
                        func=Act.Exp,
                        scale=softmax_scale,
                        bias=vb[:, kc : kc + 1],
                    )
                    if kc == 0:
                        nc.vector.tensor_copy(out=rsum, in_=expt)
                    else:
                        nc.vector.tensor_add(rsum, rsum, expt)
                    vh, koff = divmod(kc, kph)
                    for ec in range(2):
                        nc.tensor.matmul(
                            ut_a[:, ec, :],
                            vnat_h[vh][:, koff, ec * P : (ec + 1) * P],
                            expt,
                            start=(kc == 0),
                            stop=(kc == nkc - 1),
                        )
                    if kc == 1 and pending is not None:
                        emit_output(*pending)
                        pending = None
                # drain pass-A psum early (frees its slot for the next block)
                nc.vector.tensor_copy(out=un_sb[:, 0:2, :], in_=ut_a)
                rsum16 = rsum_pool.tile([P, KB], f16, tag="rsum16")
                nc.vector.tensor_copy(out=rsum16, in_=rsum)

                def emit_rowsum():
                    # partition reduce as a 1-cyc/row fp16 matmul, then
                    # transpose 128-query chunks to partitions BEFORE the
                    # reciprocal so it runs 128-lane parallel
                    rs_ps = ps_small.tile([1, KB], f32, tag="ps_small")
                    nc.tensor.matmul(rs_ps, ones_col, rsum16, start=True, stop=True)
                    rs16 = stat_pool.tile([1, KB], f16, tag="rs16")
                    nc.vector.tensor_copy(out=rs16, in_=rs_ps)
                    rcs = []
                    for qs in range(tpb):
                        rc_ps = ps_small.tile([P, 1], f16, tag="ps_small")
                        nc.tensor.transpose(
                            rc_ps, rs16[0:1, qs * P : (qs + 1) * P], one16[0:1, 0:1]
                        )
                        rc = stat_pool.tile([P, 1], f32, tag="rc")
                        nc.vector.reciprocal(out=rc, in_=rc_ps)
                        rcs.append(rc)
                    return rs16, rcs

                if qb == nqb - 1:
                    # last block: emit before pass B so pass B hides the DVE
                    # rsum tail and the epilogue starts immediately after
                    rs16, rcs = emit_rowsum()
                # pass B: e-chunks 2,3 over the stored exp tiles
                ut_b = ps_ut.tile([P, 2, KB], f32, tag="ut")
                for kc in range(nkc):
                    vh, koff = divmod(kc, kph)
                    for ec in range(2):
                        nc.tensor.matmul(
                            ut_b[:, ec, :],
                            vnat_h[vh][:, koff, (2 + ec) * P : (3 + ec) * P],
                            expts[kc],
                            start=(kc == 0),
                            stop=(kc == nkc - 1),
                        )
                # row-sums AFTER pass B for non-final blocks (the DVE rsum
                # adds trail the exps, so emitting this earlier would stall
                # the PE at the qb boundary)
                if qb != nqb - 1:
                    rs16, rcs = emit_rowsum()
                # drain pass-B psum, split DVE/ACT
                nc.vector.tensor_copy(out=un_sb[:, 2:3, :], in_=ut_b[:, 0:1, :])
                nc.scalar.copy(out=un_sb[:, 3:4, :], in_=ut_b[:, 1:2, :])
                pending = (qb, un_sb, rs16, rcs)
            emit_output(*pending)

    nc.compile()
    return nc


_CACHE = {}


def _get_nc():
    if "nc" not in _CACHE:
        _CACHE["nc"] = build_attention()
    return _CACHE["nc"]


def _in_maps(query, key, value, Wq, bq, Wk, bk, Wv, bv, n_cores=NCORES):
    Wq = np.asarray(Wq, np.float32)
    Wk = np.asarray(Wk, np.float32)
    Wv = np.asarray(Wv, np.float32)
    bq = np.asarray(bq, np.float32)
    bv = np.asarray(bv, np.float32)
    g16 = (Wk.T @ Wq).astype(np.float16)
    wvt = np.ascontiguousarray(Wv.T).astype(np.float16)
    scale = 1.0 / math.sqrt(D)
    w1 = (scale * (Wk.T @ bq)).astype(np.float16)  # [D]
    w1b = np.ascontiguousarray(np.broadcast_to(w1[None, :], (P, D)))
    bv128 = np.ascontiguousarray(
        np.broadcast_to(bv.astype(np.float16)[None, :], (P, D))
    )
    q16 = np.asarray(query, np.float16)
    k16 = np.asarray(key, np.float16)
    v16 = np.asarray(value, np.float16)
    return [
        {
            "q16": q16[i],
            "k16": k16[i],
            "v16": v16[i],
            "g16": g16,
            "wvt": wvt,
            "w1b": w1b,
            "bv128": bv128,
        }
        for i in range(n_cores)
    ]


def _build_runner():
    """Compile once and return a callable(in_maps) -> [out per core].

    Same lowering as concourse.bass2jax.run_bass_via_pjrt, but the
    jitted shard_map executable is cached so repeat kernel() calls skip
    retracing/recompiling.
    """
    import jax
    import concourse.mybir as mybir
    from concourse import bass2jax
    from jax.experimental.shard_map import shard_map
    from jax.sharding import Mesh, PartitionSpec

    bass2jax.install_neuronx_cc_hook()
    nc = _get_nc()
    partition_name = nc.partition_id_tensor.name if nc.partition_id_tensor else None
    in_names, out_names, out_avals, zero_templates = [], [], [], []
    for alloc in nc.m.functions[0].allocations:
        if not isinstance(alloc, mybir.MemoryLocationSet):
            continue
        name = alloc.memorylocations[0].name
        if alloc.kind == "ExternalInput":
            if name != partition_name:
                in_names.append(name)
        elif alloc.kind == "ExternalOutput":
            shape = tuple(alloc.tensor_shape)
            dtype = mybir.dt.np(alloc.dtype)
            out_names.append(name)
            out_avals.append(jax.core.ShapedArray(shape, dtype))
            zero_templates.append((shape, dtype))
    n_params = len(in_names)
    n_outs = len(out_names)
    all_in_names = list(in_names) + list(out_names)
    if partition_name is not None:
        all_in_names.append(partition_name)
    donate = tuple(range(n_params, n_params + n_outs))

    def _body(*args):
        operands = list(args)
        if partition_name is not None:
            operands.append(bass2jax.partition_id_tensor())
        outs = bass2jax._bass_exec_p.bind(
            *operands,
            out_avals=tuple(out_avals),
            in_names=tuple(all_in_names),
            out_names=tuple(out_names),
            lowering_input_output_aliases=(),
            sim_require_finite=True,
            sim_require_nnan=True,
            nc=nc,
        )
        return tuple(outs)

    devices = jax.devices()[:NCORES]
    mesh = Mesh(np.asarray(devices), ("core",))
    in_specs = (PartitionSpec("core"),) * (n_params + n_outs)
    out_specs = (PartitionSpec("core"),) * n_outs
    sharded = jax.jit(
        shard_map(
            _body, mesh=mesh, in_specs=in_specs, out_specs=out_specs, check_rep=False
        ),
        donate_argnums=donate,
        keep_unused=True,
    )

    def run(in_maps):
        concat_in = [
            np.concatenate([np.asarray(m[name]) for m in in_maps], axis=0)
            for name in in_names
        ]
        concat_zeros = [
            np.zeros((NCORES * shp[0], *shp[1:]), dt) for shp, dt in zero_templates
        ]
        out_arrs = sharded(*concat_in, *concat_zeros)
        out = np.asarray(out_arrs[out_names.index("out")])
        return out.reshape(NCORES, S, D)

    return run


def _get_runner():
    if "run" not in _CACHE:
        _CACHE["run"] = _build_runner()
    return _CACHE["run"]


def kernel(query, key, value, Wq, bq, Wk, bk, Wv, bv):
    run = _get_runner()
    in_maps = _in_maps(query, key, value, Wq, bq, Wk, bk, Wv, bv)
    return run(in_maps)


# revision 42
# speedup vs baseline: 1.0081x; 1.0081x over previous
"""Fused attention layer (QKV projections + softmax(QK^T/sqrt(d))V) for
Trainium2, data-parallel over the batch across 8 NeuronCores.

Projection-free formulation (per core, one batch element, S=4096, D=512):
  scores^T = key (Wk^T Wq) query^T + v[k] 1^T + 1 u[q]^T (+ const); the
  per-query additive terms cancel in softmax, so only the per-key bias
  v = key (Wk^T bq) survives and rides through the ACT exp's per-partition
  bias.  With G = Wk^T Wq folded on the HOST and applied to the key side
  (KG^T = G^T key^T), the query projection disappears entirely.  On the
  value side, out = attn value Wv^T + bv (attn rows sum to one), so value
  is consumed in its natural layout; U^T = value^T exp^T accumulates on
  PSUM in two e-chunk passes, and Wv^T is applied per 128-query tile at
  the end.  The bias enters as rowsum[q]*bv via a K=1 matmul so the final
  1/rowsum ACT scaling leaves exactly +bv.

This version removes nearly all non-GEMM work from the PE:
  - query/key/value are staged as fp16 in DRAM (host cast, identical
    rounding to the previous on-device cast); query^T is delivered by the
    DMA XBAR transpose (sync queue only, <=2048-row calls, token-fenced
    from all normal DMAs -- concurrent/cross-queue use corrupts), key^T
    by PE transposes off an early natural-layout key load, pipelined with
    the KG matmuls.
  - G = Wk^T Wq is folded on the host (weights-only prep).
  - the per-key bias v = key @ (scale Wk^T bq) is a GPSIMD multiply plus
    DVE free-dim reduce over the natural-layout key copy.
  - the output bias bv is a DVE broadcast-add (attn rows sum to one), so
    no K=1 bias matmuls except on the final block where the rowsum*bv
    psum trick keeps the tail chain PE-resident.
  - the softmax reciprocal runs after the PE transpose in [128,1] layout
    (128-lane parallel) instead of on a single partition.
All matmul operands fp16 (1 cyc/row, ~6e-4 rel err), accumulation fp32.
"""

import math

import numpy as np

S, D, P = 4096, 512, 128
NCORES = 8
KB = 512  # q block width


def build_attention(s=S, d=D, num_devices=NCORES):
    from contextlib import ExitStack

    import concourse.mybir as mybir
    import concourse.tile as tile
    from concourse import bacc
    from concourse.masks import make_identity

    f32 = mybir.dt.float32
    f16 = mybir.dt.float16
    Act = mybir.ActivationFunctionType
    Alu = mybir.AluOpType

    dc = d // P        # d/e chunks (4)
    nkc = s // P       # key chunks (32)
    nqb = s // KB      # q blocks (8)
    tpb = KB // P      # 128-sub-blocks per block (4)
    nh = 2 if nqb >= 2 else 1  # sequence halves (XBAR <=2048-row calls)
    sh = s // nh
    bph = nqb // nh    # q blocks per half
    kph = nkc // nh    # key chunks per half
    softmax_scale = 1.0 / math.sqrt(d)

    nc = bacc.Bacc(
        "TRN2", target_bir_lowering=False, debug=False, num_devices=num_devices
    )

    q16_d = nc.dram_tensor("q16", [s, d], f16, kind="ExternalInput").ap()
    k16_d = nc.dram_tensor("k16", [s, d], f16, kind="ExternalInput").ap()
    v16_d = nc.dram_tensor("v16", [s, d], f16, kind="ExternalInput").ap()
    g16_d = nc.dram_tensor("g16", [d, d], f16, kind="ExternalInput").ap()
    wvt_d = nc.dram_tensor("wvt", [d, d], f16, kind="ExternalInput").ap()
    w1b_d = nc.dram_tensor("w1b", [P, d], f16, kind="ExternalInput").ap()
    bv_d = nc.dram_tensor("bv128", [P, d], f16, kind="ExternalInput").ap()
    out_d = nc.dram_tensor("out", [s, d], f32, kind="ExternalOutput").ap()

    with tile.TileContext(nc) as tc, ExitStack() as stack:
        consts = stack.enter_context(tc.tile_pool(name="consts", bufs=1))

        one16 = consts.tile([1, 1], f16, name="one16")
        nc.vector.memset(one16, 1.0)
        ident16 = consts.tile([P, P], f16, name="ident16")
        make_identity(nc, ident16)
        ones_col = consts.tile([P, 1], f16, name="ones_col")
        nc.vector.memset(ones_col, 1.0)

        gsb = consts.tile([P, dc, d], f16, name="g_sb")       # G = Wk^T Wq
        wvt = consts.tile([P, dc, d], f16, name="wvt_sb")
        w1b = consts.tile([P, d], f16, name="w1b_sb")         # bcast scale*Wk^T bq
        bv128 = consts.tile([P, d], f16, name="bv128_sb")     # bcast bv

        # persistent activations (query^T / value split into sequence-half
        # tiles for finer DMA->compute dependency granularity)
        qryt_h = [
            consts.tile([P, dc, sh], f16, name=f"qryt_sb{i}") for i in range(nh)
        ]
        kgt = consts.tile([P, dc, s], f16, name="kgt_sb")     # (key G)^T [d', n]
        vnat_h = [
            consts.tile([P, kph, d], f16, name=f"vnat_sb{i}") for i in range(nh)
        ]
        vb = consts.tile([P, nkc], f32, name="vb_sb")         # scale * key@w1

        ps_st = stack.enter_context(tc.tile_pool(name="ps_st", bufs=2, space="PSUM"))

        # ---------------- Phase 1: loads + KG^T (keyt/knat transient) ----------
        with (
            tc.tile_pool(name="keyt_pool", bufs=1) as keyt_pool,
            tc.tile_pool(name="knat_pool", bufs=1) as knat_pool,
            tc.tile_pool(name="vbs_pool", bufs=1) as vbs_pool,
            tc.tile_pool(name="pt_ps", bufs=4, space="PSUM") as pt_ps,
            tc.tile_pool(name="kg_ps2", bufs=2, space="PSUM") as kg_ps2,
        ):
            knat = knat_pool.tile([P, nkc, d], f16, name="knat_sb")
            scratch2 = [
                vbs_pool.tile([P, d], f16, name=f"vb_scratch{i}") for i in range(8)
            ]
            keyt_h = [
                keyt_pool.tile([P, dc, sh], f16, name=f"keyt_sb{i}")
                for i in range(nh)
            ]

            # XBAR DMA-transpose discipline (hardware bug workarounds, verified
            # by probing): XBAR transposes corrupt when they execute
            # concurrently with normal DMAs (either queue) or with transposes
            # on the other queue.  So: key^T is built by PE transposes from an
            # early natural-layout key load, and only query^T (8 calls, <=2048
            # source rows each) uses the XBAR -- all on the sync queue, with
            # tiny DVE "token" copies creating data dependencies that keep
            # every normal DMA strictly outside the transpose windows.

            # group 1: natural-layout key (first pieces first, so the PE keyt
            # transposes start as early as possible) + the weights needed
            # before the epilogue; wvt is deferred to the value group
            if nkc >= 32:
                # fine 2-chunk leading pieces: the first PE transposes start
                # as soon as 0.25 MB lands, then coarser trailing pieces
                pieces = [
                    (0, 2), (2, 4), (4, 6), (6, 8),
                    (8, 12), (12, 16), (16, 24), (24, 32),
                ]
            else:
                pieces = [(i, min(i + 2, nkc)) for i in range(0, nkc, 2)]
            nc.sync.dma_start(
                out=knat[:, pieces[0][0] : pieces[0][1], :],
                in_=k16_d[pieces[0][0] * P : pieces[0][1] * P, :].rearrange(
                    "(s p) d -> p s d", p=P
                ),
            )
            nc.scalar.dma_start(
                out=knat[:, pieces[1][0] : pieces[1][1], :],
                in_=k16_d[pieces[1][0] * P : pieces[1][1] * P, :].rearrange(
                    "(s p) d -> p s d", p=P
                ),
            )
            nc.sync.dma_start(out=gsb, in_=g16_d.rearrange("(c p) e -> p c e", p=P))
            nc.scalar.dma_start(out=w1b, in_=w1b_d)
            nc.scalar.dma_start(out=bv128, in_=bv_d)
            for j, (a, b) in enumerate(pieces[2:]):
                eng = nc.sync if j % 2 == 0 else nc.scalar
                eng.dma_start(
                    out=knat[:, a:b, :],
                    in_=k16_d[a * P : b * P, :].rearrange("(s p) d -> p s d", p=P),
                )
            # token A: the qryt h0 XBAR transposes wait for all group-1 DMAs.
            # Tokens are 8-byte SBUF->SBUF micro-DMAs on the sync queue (the
            # queue the XBARs live on): a token on a compute engine becomes a
            # head-of-line block for everything paced behind it.
            qtok = qryt_h[0][0:1, :, 0:2]
            nc.sync.dma_start(out=qtok, in_=gsb[0:1, :, 0:2])
            nc.sync.dma_start(out=qtok, in_=w1b[0:1, 0:8])
            nc.sync.dma_start(out=qtok, in_=bv128[0:1, 0:8])
            for a, b in pieces:
                nc.sync.dma_start(out=qtok, in_=knat[0:1, a : a + 1, 0:8])
            # group 2: query^T first half on the XBAR (sync queue only)
            for c in range(dc):
                nc.sync.dma_start(
                    out=qryt_h[0][:, c, :],
                    in_=q16_d[0:sh, c * P : (c + 1) * P],
                    transpose=True,
                )
            # token B: natural loads wait for the qryt h0 transposes
            for i in range(nh):
                nc.sync.dma_start(
                    out=vnat_h[i][0:1, 0:4, 0:2], in_=qryt_h[0][0:1, :, 0:2]
                )
            nc.sync.dma_start(out=wvt[0:1, :, 0:2], in_=qryt_h[0][0:1, :, 0:2])
            # group 3: natural-layout value + deferred Wv^T weight.  All on
            # the SYNC queue: a gated DMA-issue on the scalar queue would
            # block the ACT engine's copy/drain/exp stream (one FIFO per
            # engine), which stalled the scores pipeline by ~8us.
            for i in range(nh):
                nc.sync.dma_start(
                    out=vnat_h[i],
                    in_=v16_d[i * sh : (i + 1) * sh, :].rearrange(
                        "(s p) d -> p s d", p=P
                    ),
                )
            nc.sync.dma_start(out=wvt, in_=wvt_d.rearrange("(c p) e -> p c e", p=P))
            if nh > 1:
                # token C: qryt h1 transposes wait for the natural loads
                for i in range(nh):
                    nc.sync.dma_start(
                        out=qryt_h[1][0:1, :, 0:2], in_=vnat_h[i][0:1, 0:4, 0:2]
                    )
                nc.sync.dma_start(
                    out=qryt_h[1][0:1, :, 0:2], in_=wvt[0:1, :, 0:2]
                )
                # group 4: query^T second half on the XBAR
                for c in range(dc):
                    nc.sync.dma_start(
                        out=qryt_h[1][:, c, :],
                        in_=q16_d[sh:s, c * P : (c + 1) * P],
                        transpose=True,
                    )

            # key^T per block via PE transposes from knat (psum -> ACT copy),
            # software-pipelined one block ahead of the KG matmuls; the DVE
            # v-vector reduces v[k] = key @ (scale Wk^T bq) trail each block
            cpb = nkc // nqb  # key chunks per q-sized block (4)

            def emit_keyt_block(nb):
                # two 128-key sub-blocks per psum tile: halves the copy
                # instruction count and the per-block cross-engine latency
                # in the transpose -> copy -> KG pipeline
                kh, nboff = divmod(nb, bph)
                for pi in range(cpb // 2):
                    pt = pt_ps.tile([P, dc, 2 * P], f16, tag="pt")
                    for sp in range(2):
                        si = 2 * pi + sp
                        for c in range(dc):
                            nc.tensor.transpose(
                                pt[:, c, sp * P : (sp + 1) * P],
                                knat[:, nb * cpb + si, c * P : (c + 1) * P],
                                ident16,
                            )
                    a = (nboff * cpb + 2 * pi) * P
                    dst = keyt_h[kh][:, :, a : a + 2 * P]
                    if pi == 1:
                        nc.vector.tensor_copy(out=dst, in_=pt)
                    else:
                        nc.scalar.copy(out=dst, in_=pt)

            def emit_kg_block(nb):
                # the last two blocks use a dedicated psum pool so the scores
                # pipeline's psum_st slots never wait on late KG drains
                pool, tag = (
                    (kg_ps2, "kg2") if nb >= nqb - 2 else (ps_st, "psum_st")
                )
                kh, nboff = divmod(nb, bph)
                for ec in range(dc):
                    pp = pool.tile([P, KB], f32, tag=tag)
                    for c in range(dc):
                        nc.tensor.matmul(
                            pp,
                            gsb[:, c, ec * P : (ec + 1) * P],
                            keyt_h[kh][:, c, nboff * KB : (nboff + 1) * KB],
                            start=(c == 0),
                            stop=(c == dc - 1),
                        )
                    if ec % 2:
                        nc.scalar.copy(
                            out=kgt[:, ec, nb * KB : (nb + 1) * KB], in_=pp
                        )
                    else:
                        nc.vector.tensor_copy(
                            out=kgt[:, ec, nb * KB : (nb + 1) * KB], in_=pp
                        )
            def emit_vb(i):
                scratch = scratch2[i % 8]
                nc.gpsimd.tensor_mul(scratch, knat[:, i, :], w1b)
                nc.vector.tensor_reduce(
                    out=vb[:, i : i + 1],
                    in_=scratch,
                    axis=mybir.AxisListType.X,
                    op=Alu.add,
                )

            # v[k] = key @ (scale Wk^T bq): multiply on the (otherwise idle)
            # GPSIMD, free-dim reduce on the DVE, one chunk per KG block with
            # the remainder after the loop
            emit_keyt_block(0)
            for nb in range(1, nqb):
                emit_keyt_block(nb)
                emit_kg_block(nb - 1)
                emit_vb(nb - 1)
            emit_kg_block(nqb - 1)
            for i in range(nqb - 1, nkc):
                emit_vb(i)

        # ---------------- Phase 2: attention (scores transposed) ----------------
        with (
            tc.tile_pool(name="ps_small", bufs=2, space="PSUM") as ps_small,
            tc.tile_pool(name="expt_pool", bufs=nkc) as expt_pool,
            tc.tile_pool(name="rsum_pool", bufs=2) as rsum_pool,
            tc.tile_pool(name="unsb_pool", bufs=2) as unsb_pool,
            tc.tile_pool(name="osb_pool", bufs=2) as osb_pool,
            tc.tile_pool(name="stat_pool", bufs=8) as stat_pool,
            tc.tile_pool(name="ps_ut", bufs=2, space="PSUM") as ps_ut,
        ):

            def emit_output(qb, un_sb, rs16, rcs):
                for qs in range(tpb):
                    last = qb == nqb - 1
                    po = ps_small.tile([P, d], f32, tag="ps_small")
                    for c in range(dc):
                        nc.tensor.matmul(
                            po,
                            un_sb[:, c, qs * P : (qs + 1) * P],
                            wvt[:, c, :],
                            start=(c == 0),
                            stop=(c == dc - 1) and not last,
                        )
                    if last:
                        # rowsum[q]*bv rides the psum so the final 1/rowsum
                        # scaling leaves exactly +bv with no DVE op in the
                        # critical tail chain
                        nc.tensor.matmul(
                            po,
                            rs16[0:1, qs * P : (qs + 1) * P],
                            bv128[0:1, :],
                            start=False,
                            stop=True,
                        )
                    out_sb = osb_pool.tile([P, d], f32, tag="out_sb")
                    if qb == 0 and qs == 0:
                        # token D: the first output DMA (a normal DMA) must not
                        # overlap the trailing qryt transposes
                        nc.gpsimd.tensor_copy(
                            out=out_sb[0:1, 0:2],
                            in_=qryt_h[nh - 1][0:1, 0:1, sh - 2 : sh],
                        )
                    # out = po/rowsum + bv (attn rows sum to one, so the bias
                    # needs no rowsum compensation): scale on ACT, bias on DVE
                    if last and qs % 2:
                        nc.vector.tensor_scalar_mul(out_sb, po, rcs[qs][:, 0:1])
                    else:
                        nc.scalar.activation(
                            out=out_sb,
                            in_=po,
                            func=Act.Identity,
                            scale=rcs[qs][:, 0:1],
                        )
                    if not last:
                        nc.vector.tensor_add(out_sb, out_sb, bv128)
                    nc.sync.dma_start(
                        out=out_d[qb * KB + qs * P : qb * KB + (qs + 1) * P, :],
                        in_=out_sb,
                    )

            pending = None
            for qb in range(nqb):
                rsum = rsum_pool.tile([P, KB], f32, tag="rsum")
                ut_a = ps_ut.tile([P, 2, KB], f32, tag="ut")
                un_sb = unsb_pool.tile([P, dc, KB], f16, tag="un_sb")
                expts = []
                for kc in range(nkc):
                    psum_st = ps_st.tile([P, KB], f32, tag="psum_st")
                    qh, qoff = divmod(qb, bph)
                    for ec in range(dc):
                        nc.tensor.matmul(
                            psum_st,
                            kgt[:, ec, kc * P : (kc + 1) * P],
                            qryt_h[qh][:, ec, qoff * KB : (qoff + 1) * KB],
                            start=(ec == 0),
                            stop=(ec == dc - 1),
                        )
                    expt = expt_pool.tile([P, KB], f16, tag="expt")
                    expts.append(expt)
                    nc.scalar.activation(
                        out=expt,
                        in_=psum_st,
                        func=Act.Exp,
                        scale=softmax_scale,
                        bias=vb[:, kc : kc + 1],
                    )
                    if kc == 0:
                        nc.vector.tensor_copy(out=rsum, in_=expt)
                    else:
                        nc.vector.tensor_add(rsum, rsum, expt)
                    vh, koff = divmod(kc, kph)
                    for ec in range(2):
                        nc.tensor.matmul(
                            ut_a[:, ec, :],
                            vnat_h[vh][:, koff, ec * P : (ec + 1) * P],
                            expt,
                            start=(kc == 0),
                            stop=(kc == nkc - 1),
                        )
                    if kc == 1 and pending is not None:
                        emit_output(*pending)
                        pending = None
                # drain pass-A psum early (frees its slot for the next block)
                nc.vector.tensor_copy(out=un_sb[:, 0:2, :], in_=ut_a)
                rsum16 = rsum_pool.tile([P, KB], f16, tag="rsum16")
                nc.vector.tensor_copy(out=rsum16, in_=rsum)

                def emit_rowsum():
                    # partition reduce as a 1-cyc/row fp16 matmul, then
                    # transpose 128-query chunks to partitions BEFORE the
                    # reciprocal so it runs 128-lane parallel
                    rs_ps = ps_small.tile([1, KB], f32, tag="ps_small")
                    nc.tensor.matmul(rs_ps, ones_col, rsum16, start=True, stop=True)
                    rs16 = stat_pool.tile([1, KB], f16, tag="rs16")
                    nc.vector.tensor_copy(out=rs16, in_=rs_ps)
                    rcs = []
                    for qs in range(tpb):
                        rc_ps = ps_small.tile([P, 1], f16, tag="ps_small")
                        nc.tensor.transpose(
                            rc_ps, rs16[0:1, qs * P : (qs + 1) * P], one16[0:1, 0:1]
                        )
                        rc = stat_pool.tile([P, 1], f32, tag="rc")
                        nc.vector.reciprocal(out=rc, in_=rc_ps)
                        rcs.append(rc)
                    return rs16, rcs

                if qb == nqb - 1:
                    # last block: emit before pass B so pass B hides the DVE
                    # rsum tail and the epilogue starts immediately after
                    rs16, rcs = emit_rowsum()
                # pass B: e-chunks 2,3 over the stored exp tiles
                ut_b = ps_ut.tile([P, 2, KB], f32, tag="ut")
                for kc in range(nkc):
                    vh, koff = divmod(kc, kph)
                    for ec in range(2):
                        nc.tensor.matmul(
                            ut_b[:, ec, :],
                            vnat_h[vh][:, koff, (2 + ec) * P : (3 + ec) * P],
                            expts[kc],
                            start=(kc == 0),
                            stop=(kc == nkc - 1),
                        )
                # row-sums AFTER pass B for non-final blocks (the DVE rsum
                # adds trail the exps, so emitting this earlier would stall
                # the PE at the qb boundary)
                if qb != nqb - 1:
                    rs16, rcs = emit_rowsum()
                # drain pass-B psum, split DVE/ACT
                nc.vector.tensor_copy(out=un_sb[:, 2:3, :], in_=ut_b[:, 0:1, :])
                nc.scalar.copy(out=un_sb[:, 3:4, :], in_=ut_b[:, 1:2, :])
                pending = (qb, un_sb, rs16, rcs)
            emit_output(*pending)

    nc.compile()
    return nc


_CACHE = {}


def _get_nc():
    if "nc" not in _CACHE:
        _CACHE["nc"] = build_attention()
    return _CACHE["nc"]


def _in_maps(query, key, value, Wq, bq, Wk, bk, Wv, bv, n_cores=NCORES):
    Wq = np.asarray(Wq, np.float32)
    Wk = np.asarray(Wk, np.float32)
    Wv = np.asarray(Wv, np.float32)
    bq = np.asarray(bq, np.float32)
    bv = np.asarray(bv, np.float32)
    g16 = (Wk.T @ Wq).astype(np.float16)
    wvt = np.ascontiguousarray(Wv.T).astype(np.float16)
    scale = 1.0 / math.sqrt(D)
    w1 = (scale * (Wk.T @ bq)).astype(np.float16)  # [D]
    w1b = np.ascontiguousarray(np.broadcast_to(w1[None, :], (P, D)))
    bv128 = np.ascontiguousarray(
        np.broadcast_to(bv.astype(np.float16)[None, :], (P, D))
    )
    q16 = np.asarray(query, np.float16)
    k16 = np.asarray(key, np.float16)
    v16 = np.asarray(value, np.float16)
    return [
        {
            "q16": q16[i],
            "k16": k16[i],
            "v16": v16[i],
            "g16": g16,
            "wvt": wvt,
            "w1b": w1b,
            "bv128": bv128,
        }
        for i in range(n_cores)
    ]


def _build_runner():
    """Compile once and return a callable(in_maps) -> [out per core].

    Same lowering as concourse.bass2jax.run_bass_via_pjrt, but the
    jitted shard_map executable is cached so repeat kernel() calls skip
    retracing/recompiling.
    """
    import jax
    import concourse.mybir as mybir
    from concourse import bass2jax
    from jax.experimental.shard_map import shard_map
    from jax.sharding import Mesh, PartitionSpec

    bass2jax.install_neuronx_cc_hook()
    nc = _get_nc()
    partition_name = nc.partition_id_tensor.name if nc.partition_id_tensor else None
    in_names, out_names, out_avals, zero_templates = [], [], [], []
    for alloc in nc.m.functions[0].allocations:
        if not isinstance(alloc, mybir.MemoryLocationSet):
            continue
        name = alloc.memorylocations[0].name
        if alloc.kind == "ExternalInput":
            if name != partition_name:
                in_names.append(name)
        elif alloc.kind == "ExternalOutput":
            shape = tuple(alloc.tensor_shape)
            dtype = mybir.dt.np(alloc.dtype)
            out_names.append(name)
            out_avals.append(jax.core.ShapedArray(shape, dtype))
            zero_templates.append((shape, dtype))
    n_params = len(in_names)
    n_outs = len(out_names)
    all_in_names = list(in_names) + list(out_names)
    if partition_name is not None:
        all_in_names.append(partition_name)
    donate = tuple(range(n_params, n_params + n_outs))

    def _body(*args):
        operands = list(args)
        if partition_name is not None:
            operands.append(bass2jax.partition_id_tensor())
        outs = bass2jax._bass_exec_p.bind(
            *operands,
            out_avals=tuple(out_avals),
            in_names=tuple(all_in_names),
            out_names=tuple(out_names),
            lowering_input_output_aliases=(),
            sim_require_finite=True,
            sim_require_nnan=True,
            nc=nc,
        )
        return tuple(outs)

    devices = jax.devices()[:NCORES]
    mesh = Mesh(np.asarray(devices), ("core",))
    in_specs = (PartitionSpec("core"),) * (n_params + n_outs)
    out_specs = (PartitionSpec("core"),) * n_outs
    sharded = jax.jit(
        shard_map(
            _body, mesh=mesh, in_specs=in_specs, out_specs=out_specs, check_rep=False
        ),
        donate_argnums=donate,
        keep_unused=True,
    )

    def run(in_maps):
        concat_in = [
            np.concatenate([np.asarray(m[name]) for m in in_maps], axis=0)
            for name in in_names
        ]
        concat_zeros = [
            np.zeros((NCORES * shp[0], *shp[1:]), dt) for shp, dt in zero_templates
        ]
        out_arrs = sharded(*concat_in, *concat_zeros)
        out = np.asarray(out_arrs[out_names.index("out")])
        return out.reshape(NCORES, S, D)

    return run


def _get_runner():
    if "run" not in _CACHE:
        _CACHE["run"] = _build_runner()
    return _CACHE["run"]


def kernel(query, key, value, Wq, bq, Wk, bk, Wv, bv):
    run = _get_runner()
    in_maps = _in_maps(query, key, value, Wq, bq, Wk, bk, Wv, bv)
    return run(in_maps)
